# revision 1
# baseline (speedup 1.0000x reference)
"""Self-contained TRN2 Bass kernel for the RGCN message-passing problem.

kernel(**inputs) takes the FULL unsharded inputs (text, src, dst, rel,
bases, comp, bias), shards edges by destination window across the 8
NeuronCores, runs the SPMD Bass program via run_bass_kernel_spmd, and
returns the full [64, 512, 256] float32 output.
"""

import numpy as np
import ml_dtypes

import concourse.bass as bass
import concourse.tile as tile
from concourse import bacc, mybir

F = 256      # in features
O = 256      # out features
NB = 3       # bases
WINDOW = 64  # dst rows per window
GBUFS = 6    # gather tile buffering depth (feeds 4 SWDGE queues)
NQ = 4       # SWDGE queues


def plan_calls(slot_cws, cpc):
    """Split each window slot into gather calls of <= cpc chunks.
    Returns list of (slot, chunk_lo, n_chunks) in execution order."""
    calls = []
    for i, cw in enumerate(slot_cws):
        lo = 0
        while lo < cw:
            n = min(cpc, cw - lo)
            calls.append((i, lo, n))
            lo += n
    return calls


def build_program(n_nodes, slot_cws, cpc=8, n_cores=8):
    slot_cws = list(slot_cws)
    nw = len(slot_cws)
    nchunks = sum(slot_cws)
    epad = nchunks * 128
    dcore = nw * WINDOW
    calls = plan_calls(slot_cws, cpc)
    ncalls = len(calls)

    bf16 = mybir.dt.bfloat16
    f32 = mybir.dt.float32
    i16 = mybir.dt.int16
    i32 = mybir.dt.int32

    # bf16 DRAM I/O breaks NEFF load under the PJRT path; all bf16 payloads
    # travel as int16 containers and are bitcast on-chip.
    nc = bacc.Bacc("TRN2", target_bir_lowering=False, debug=False,
                   num_devices=n_cores, num_swdge_queues=NQ)
    h_d = nc.dram_tensor("h", [n_nodes, F], i16, kind="ExternalInput").ap()
    gidx_d = nc.dram_tensor("gidx", [128, epad // 16], i16,
                            kind="ExternalInput").ap()
    w1h_d = nc.dram_tensor("w1h", [128, nchunks, NB * WINDOW], i16,
                           kind="ExternalInput").ap()
    cnt_d = nc.dram_tensor("cnt", [1, ncalls], i32, kind="ExternalInput").ap()
    bases_d = nc.dram_tensor("bases", [NB, F, O], i16,
                             kind="ExternalInput").ap()
    bias_d = nc.dram_tensor("bias", [1, O], i16, kind="ExternalInput").ap()
    out_d = nc.dram_tensor("out", [dcore, O], i16, kind="ExternalOutput").ap()

    with tile.TileContext(nc) as tc:
        with (
            tc.tile_pool(name="const", bufs=1) as cpool,
            tc.tile_pool(name="gather", bufs=GBUFS) as gpool,
            tc.tile_pool(name="w1h", bufs=4) as wpool,
            tc.tile_pool(name="abt", bufs=2) as apool,
            tc.tile_pool(name="ost", bufs=2) as opool,
            tc.tile_pool(name="ps1", bufs=2, space="PSUM") as ps1,
            tc.tile_pool(name="ps2", bufs=2, space="PSUM") as ps2,
        ):
            # ---- prologue ----
            gidx_sb = cpool.tile([128, epad // 16], i16)
            # call 0's slice first so the gather pipeline starts immediately
            ntot = epad // 16
            cuts = [0, calls[0][2] * 8]
            cuts += [cuts[1] + (ntot - cuts[1]) * k // 3 for k in (1, 2, 3)]
            for lo, hi in zip(cuts[:-1], cuts[1:]):
                if hi > lo:
                    nc.sync.dma_start(gidx_sb[:, lo:hi], gidx_d[:, lo:hi])
            bases_i = cpool.tile([128, NB, 2, O], i16)
            for b in range(NB):
                for h in range(2):
                    nc.sync.dma_start(bases_i[:, b, h, :],
                                      bases_d[b, h * 128:(h + 1) * 128, :])
            bias_i = cpool.tile([1, O], i16)
            nc.sync.dma_start(bias_i[:], bias_d[:])
            bias_sb = bias_i[:].bitcast(bf16)
            ones_sb = cpool.tile([1, WINDOW], bf16)
            nc.vector.memset(ones_sb[:], 1.0)

            # ---- main pipeline ----
            p1 = [None, None]
            chunk_base = 0
            for j, (slot, clo, ncall) in enumerate(calls):
                nidx = ncall * 128
                G = gpool.tile([128, cpc, F], i16, tag="G", name="G")
                # pads carry index 0 (a real row; W1h zeroes their weights),
                # so every gathered row is valid and no trim/memset is needed
                nc.gpsimd.dma_gather(
                    G[:, 0:ncall, :], h_d[:],
                    gidx_sb[:, chunk_base * 8:(chunk_base + ncall) * 8],
                    nidx, nidx, F, queue_num=j % NQ)
                W = wpool.tile([128, cpc, NB * WINDOW], i16, tag="W", name="W")
                nc.sync.dma_start(
                    W[:, 0:ncall, :],
                    w1h_d[:, chunk_base:chunk_base + ncall, :])
                for c in range(ncall):
                    cw_pos = clo + c
                    if cw_pos == 0:
                        p1 = [ps1.tile([128, NB * WINDOW], f32,
                                       tag=f"p1h{h}", name=f"p1h{h}")
                              for h in range(2)]
                    last = (cw_pos == slot_cws[slot] - 1)
                    for h in range(2):
                        nc.tensor.matmul(
                            p1[h][:],
                            G[:, c, h * 128:(h + 1) * 128].bitcast(bf16),
                            W[:, c, :].bitcast(bf16),
                            start=(cw_pos == 0), stop=last)
                    if last:
                        abt = apool.tile([128, 2, NB * WINDOW], bf16,
                                         tag="abt", name="abt")
                        for h in range(2):
                            nc.vector.tensor_copy(abt[:, h, :], p1[h][:])
                        p2 = ps2.tile([WINDOW, O], f32, tag="p2", name="p2")
                        nc.tensor.matmul(p2[:], ones_sb[:], bias_sb,
                                         start=True, stop=False)
                        for b in range(NB):
                            for h in range(2):
                                nc.tensor.matmul(
                                    p2[:],
                                    abt[:, h, b * WINDOW:(b + 1) * WINDOW],
                                    bases_i[:, b, h, :].bitcast(bf16),
                                    start=False,
                                    stop=(b == NB - 1 and h == 1))
                        osb = opool.tile([WINDOW, O], bf16, tag="osb",
                                         name="osb")
                        nc.scalar.activation(
                            osb[:], p2[:], mybir.ActivationFunctionType.Relu)
                        nc.sync.dma_start(
                            out_d[slot * WINDOW:(slot + 1) * WINDOW, :],
                            osb[:].bitcast(i16))
                chunk_base += ncall

    nc.compile()
    return nc


def host_prep(src, dst, rel, comp, n_nodes, n_cores, cpc=8):
    """Sort/deal/pad edges; build the streamed W1h relayout."""
    dcore = n_nodes // n_cores
    nw = dcore // WINDOW
    ngw = n_cores * nw
    w_edge = comp[rel].astype(ml_dtypes.bfloat16)        # [E, NB]
    gw = (dst // WINDOW).astype(np.int64)
    order = np.argsort(gw, kind="stable")
    counts = np.bincount(gw, minlength=ngw)
    starts = np.concatenate([[0], np.cumsum(counts)])

    # deal windows to cores by descending count; slot capacity = group max
    ranked = np.argsort(-counts, kind="stable")
    slot_cws = [max(1, -(-int(counts[ranked[n_cores * i]]) // 128))
                for i in range(nw)]
    calls = plan_calls(slot_cws, cpc)
    nchunks = sum(slot_cws)
    epad = nchunks * 128
    ncalls = len(calls)

    gidx = np.zeros((n_cores, epad), np.int16)
    w1h = np.zeros((n_cores, epad, NB * WINDOW), ml_dtypes.bfloat16)
    cnts = np.zeros((n_cores, 1, ncalls), np.int32)
    win_of_slot = np.zeros((n_cores, nw), np.int64)
    dstloc = (dst % WINDOW).astype(np.int64)

    slot_base = np.zeros(nw, np.int64)
    acc = 0
    for i, cw in enumerate(slot_cws):
        slot_base[i] = acc
        acc += cw
    bidx = np.arange(NB) * WINDOW
    for k in range(n_cores):
        for i in range(nw):
            wid = int(ranked[n_cores * i + k])
            win_of_slot[k, i] = wid
            es = order[starts[wid]:starts[wid + 1]]
            base = slot_base[i] * 128
            n = len(es)
            gidx[k, base:base + n] = src[es].astype(np.int16)
            pos = base + np.arange(n)
            w1h[k, pos[:, None], bidx[None, :] + dstloc[es][:, None]] = \
                w_edge[es]
    for j, (slot, clo, ncall) in enumerate(calls):
        base = (slot_base[slot] + clo) * 128
        seg = gidx[:, base:base + ncall * 128]
        cnts[:, 0, j] = (seg >= 0).sum(axis=1)

    # wrap gidx: idx i -> partition i%16, slot i//16; replicate to 128 parts
    gidx_w = gidx.reshape(n_cores, epad // 16, 16).transpose(0, 2, 1)
    gidx_w = np.tile(gidx_w, (1, 8, 1)).copy()
    # w1h layout: edge e -> [e%128, e//128, :]
    w1h_t = w1h.reshape(n_cores, nchunks, 128, NB * WINDOW)
    w1h_t = w1h_t.transpose(0, 2, 1, 3).copy()
    return gidx_w, w1h_t, cnts, tuple(slot_cws), win_of_slot


def rgcn_kernel(text, src, dst, rel, bases, comp, bias, n_cores=8,
                run_fn=None, cpc=8, nc_cache={}):
    """Full-input kernel: shard, run on 8 cores, reassemble output."""
    Bt, St, INF = text.shape
    n_nodes = Bt * St
    h = text.reshape(n_nodes, INF)

    src = np.asarray(src).astype(np.int64)
    dst = np.asarray(dst).astype(np.int64)
    rel = np.asarray(rel).astype(np.int64)
    bases_np = np.asarray(bases, np.float32)
    comp_np = np.asarray(comp, np.float32)
    bias_np = np.asarray(bias, np.float32)

    gidx_w, w1h_t, cnts, slot_cws, win_of_slot = host_prep(
        src, dst, rel, comp_np, n_nodes, n_cores, cpc)
    key = (n_nodes, slot_cws, cpc, n_cores)
    if key not in nc_cache:
        nc_cache[key] = build_program(n_nodes, slot_cws, cpc, n_cores)
    nc = nc_cache[key]

    h_bf = np.asarray(h, np.float32).astype(ml_dtypes.bfloat16).view(np.int16)
    bases_bf = bases_np.astype(ml_dtypes.bfloat16).view(np.int16)
    bias_bf = bias_np.reshape(1, O).astype(ml_dtypes.bfloat16).view(np.int16)

    in_maps = [
        dict(h=h_bf, gidx=gidx_w[k], w1h=w1h_t[k].view(np.int16),
             cnt=cnts[k], bases=bases_bf, bias=bias_bf)
        for k in range(n_cores)
    ]
    from concourse.bass_utils import run_bass_kernel_spmd
    if run_fn is None:
        res = run_bass_kernel_spmd(nc, in_maps, list(range(n_cores)))
        outs = [res.results[k]["out"] for k in range(n_cores)]
    else:
        outs = run_fn(nc, in_maps)

    out = np.zeros((n_nodes, O), np.float32)
    nw = len(slot_cws)
    W = WINDOW
    for k in range(n_cores):
        ok = outs[k].view(ml_dtypes.bfloat16).astype(np.float32)
        for i in range(nw):
            wid = win_of_slot[k][i]
            out[wid * W:(wid + 1) * W] = ok[i * W:(i + 1) * W]
    return out.reshape(Bt, St, O)


_NC_CACHE = {}


def kernel(text, src, dst, rel, bases, comp, bias):
    out = rgcn_kernel(
        np.asarray(text, np.float32),
        np.asarray(src), np.asarray(dst), np.asarray(rel),
        np.asarray(bases, np.float32), np.asarray(comp, np.float32),
        np.asarray(bias, np.float32),
        n_cores=8, nc_cache=_NC_CACHE)
    return np.ascontiguousarray(out, np.float32)

